# revision 15
# baseline (speedup 1.0000x reference)
"""DSNT + JSD + inter-landmark-distance double loss on 8 Trainium2 NeuronCores.

Math (per heatmap of 65536 elements, b in [0,128), c in {0,1}):
  e    = exp(x)                      S = sum(e)       p = e / S
  pred = (sum(p*xs), sum(p*ys))
  q    = t + p,  m = q/2
  jsd  = mean(m*log m - m^2) = (0.5*sum(q log q) - 0.5*ln2*sum(q) - 0.25*sum(q^2)) / 65536
  ed   = |pred - true|_2 with true from argmax(t)
  loss = (sum_bc(ed + jsd) + sum_b |pd-td|/td) / B

Device computes, per heatmap, per-partition (128-way) partial stats:
  sum(e) (ACT exp accum), sum(e*xs)/sum(e*ys) partial rows (fp32 matmul or
  fused scalar_tensor_tensor), sum(q), sum(q log q), sum(q^2)
  (scalar_tensor_tensor accums), top-8 max/argmax of t (DVE max/max_index).
Host (numpy, float64) does the tiny 256-heatmap scalar epilogue.

Sharding: pure data parallel over B (16 samples -> 32 heatmaps per core).
"""

import numpy as np

B, C, H, W = 128, 2, 256, 256
NCORES = 8
SPC = B // NCORES          # samples per core = 16
HM = SPC * C               # heatmaps per core = 32
P, F = 128, 512            # SBUF tile: 128 partitions x 512 (= 65536 el)
GRP = 8                    # heatmaps per softmax-normalization group
NGRP = HM // GRP           # 4

USE_STT = True             # fused (in0 op0 s) op1 in1 + accum on DVE
USE_MM = False             # fp32 matmul path for sum(e*xs)/sum(e*ys)
USE_MM_COMBINE = False     # ones-matmul partition reduce (else gpsimd allreduce)

_CACHE = {}


def _consts():
    f32 = np.float32
    xs = ((np.arange(W, dtype=f32) + 1.0) - W / 2.0) / W
    ys = ((np.arange(H, dtype=f32) + 1.0) - H / 2.0) / H
    j = np.arange(F)
    out = {}
    if USE_MM:
        w3 = np.zeros((P, 3), dtype=f32)
        w3[:, 0] = 1.0
        w3[:, 1] = ys[2 * np.arange(P)]
        w3[:, 2] = ys[2 * np.arange(P) + 1]
        xb = np.zeros((P, F), dtype=f32)
        xb[0::32, :] = xs[j % W][None, :]
        xb[1::32, :] = (j < 256).astype(f32)[None, :]
        xb[2::32, :] = (j >= 256).astype(f32)[None, :]
        out["w3"] = w3
        out["xb"] = xb
    else:
        # full elementwise coordinate maps: element k = 512*p + j,
        # w = k % 256, h = k // 256 = 2p + (j >= 256)
        pp = np.arange(P)
        out["xmap"] = np.tile(xs[j % W], (P, 1)).astype(f32)
        hmat = 2 * pp[:, None] + (j >= 256)[None, :]
        out["ymap"] = ys[hmat].astype(f32)
    if USE_MM_COMBINE:
        out["ones1"] = np.ones((P, 1), dtype=f32)
        out["nones"] = np.full((1, P), -1.0, dtype=f32)
    return out


def _build():
    from contextlib import ExitStack

    import concourse.tile as tile
    from concourse import bacc, bass_isa, mybir

    f32 = mybir.dt.float32
    u32 = mybir.dt.uint32
    AX = mybir.AluOpType
    AF = mybir.ActivationFunctionType

    nc = bacc.Bacc("TRN2", target_bir_lowering=False, debug=False)

    x_d = nc.dram_tensor("x", [HM, P, F], f32, kind="ExternalInput").ap()
    t_d = nc.dram_tensor("t", [HM, P, F], f32, kind="ExternalInput").ap()
    if USE_MM:
        w3_d = nc.dram_tensor("w3", [P, 3], f32, kind="ExternalInput").ap()
        xb_d = nc.dram_tensor("xb", [P, F], f32, kind="ExternalInput").ap()
    else:
        xmap_d = nc.dram_tensor("xmap", [P, F], f32, kind="ExternalInput").ap()
        ymap_d = nc.dram_tensor("ymap", [P, F], f32, kind="ExternalInput").ap()
    if USE_MM_COMBINE:
        ones1_d = nc.dram_tensor("ones1", [P, 1], f32, kind="ExternalInput").ap()
        nones_d = nc.dram_tensor("nones", [1, P], f32, kind="ExternalInput").ap()

    o_se = nc.dram_tensor("o_se", [NGRP, P, GRP], f32, kind="ExternalOutput").ap()
    o_sp = nc.dram_tensor("o_sp", [NGRP, P, GRP], f32, kind="ExternalOutput").ap()
    o_sq = nc.dram_tensor("o_sq", [NGRP, P, GRP], f32, kind="ExternalOutput").ap()
    o_sql = nc.dram_tensor("o_sql", [NGRP, P, GRP], f32, kind="ExternalOutput").ap()
    o_sqq = nc.dram_tensor("o_sqq", [NGRP, P, GRP], f32, kind="ExternalOutput").ap()
    o_tmax = nc.dram_tensor("o_tmax", [NGRP, P, GRP * 8], f32, kind="ExternalOutput").ap()
    o_tidx = nc.dram_tensor("o_tidx", [NGRP, P, GRP * 8], u32, kind="ExternalOutput").ap()
    if USE_MM:
        o_red = nc.dram_tensor("o_red", [NGRP, 2, P], f32, kind="ExternalOutput").ap()
    else:
        o_sex = nc.dram_tensor("o_sex", [NGRP, P, GRP], f32, kind="ExternalOutput").ap()
        o_sey = nc.dram_tensor("o_sey", [NGRP, P, GRP], f32, kind="ExternalOutput").ap()

    with tile.TileContext(nc) as tc, ExitStack() as ctx:
        cpool = ctx.enter_context(tc.tile_pool(name="consts", bufs=1))
        xpool = ctx.enter_context(tc.tile_pool(name="xt", bufs=18))
        tpool = ctx.enter_context(tc.tile_pool(name="tt", bufs=18))
        epool = ctx.enter_context(tc.tile_pool(name="et", bufs=4))
        ppool = ctx.enter_context(tc.tile_pool(name="pt", bufs=3))
        qpool = ctx.enter_context(tc.tile_pool(name="qt", bufs=4))
        lpool = ctx.enter_context(tc.tile_pool(name="lt", bufs=3))
        dpool = ctx.enter_context(tc.tile_pool(name="dump", bufs=4))
        spool = ctx.enter_context(tc.tile_pool(name="stats", bufs=2))
        smpool = ctx.enter_context(tc.tile_pool(name="small", bufs=2))
        if USE_MM:
            rpool = ctx.enter_context(tc.tile_pool(name="rstack", bufs=2))
            ps_r = ctx.enter_context(tc.tile_pool(name="ps_r", bufs=4, space="PSUM"))
        if USE_MM_COMBINE:
            ps_s = ctx.enter_context(tc.tile_pool(name="ps_s", bufs=2, space="PSUM"))
            ps_b = ctx.enter_context(tc.tile_pool(name="ps_b", bufs=2, space="PSUM"))

        if USE_MM:
            w3 = cpool.tile([P, 3], f32)
            nc.sync.dma_start(w3[:], w3_d[:])
            xb = cpool.tile([P, F], f32)
            nc.sync.dma_start(xb[:], xb_d[:])
        else:
            xmap = cpool.tile([P, F], f32)
            nc.sync.dma_start(xmap[:], xmap_d[:])
            ymap = cpool.tile([P, F], f32)
            nc.sync.dma_start(ymap[:], ymap_d[:])
        if USE_MM_COMBINE:
            ones1 = cpool.tile([P, 1], f32)
            nc.sync.dma_start(ones1[:], ones1_d[:])
            nones = cpool.tile([1, P], f32)
            nc.sync.dma_start(nones[:], nones_d[:])

        for grp in range(NGRP):
            se = spool.tile([P, GRP], f32, tag="se", name=f"se_{grp}")
            sp = spool.tile([P, GRP], f32, tag="sp", name=f"sp_{grp}")
            sq = spool.tile([P, GRP], f32, tag="sq", name=f"sq_{grp}")
            sql = spool.tile([P, GRP], f32, tag="sql", name=f"sql_{grp}")
            sqq = spool.tile([P, GRP], f32, tag="sqq", name=f"sqq_{grp}")
            tmax = spool.tile([P, GRP * 8], f32, tag="tmax", name=f"tmax_{grp}")
            tidx = spool.tile([P, GRP * 8], u32, tag="tidx", name=f"tidx_{grp}")
            if USE_MM:
                rstacks = [rpool.tile([P, F], f32, tag=f"rstack{m}",
                                      name=f"rstack{m}_{grp}") for m in range(2)]
                reds = [smpool.tile([P, 1], f32, tag=f"red{m}",
                                    name=f"red{m}_{grp}") for m in range(2)]
                for m in range(2):
                    nc.gpsimd.memset(rstacks[m][:], 0.0)
            else:
                sex = spool.tile([P, GRP], f32, tag="sex", name=f"sex_{grp}")
                sey = spool.tile([P, GRP], f32, tag="sey", name=f"sey_{grp}")

            xts, tts = [], []
            # ---- phase A ----
            for i in range(GRP):
                g = grp * GRP + i
                xt = xpool.tile([P, F], f32, tag="xt", name=f"xt_{g}")
                nc.sync.dma_start(xt[:], x_d[g])
                tt = tpool.tile([P, F], f32, tag="tt", name=f"tt_{g}")
                nc.sync.dma_start(tt[:], t_d[g])
                xts.append(xt)
                tts.append(tt)

                et = epool.tile([P, F], f32, tag="et", name=f"et_{g}")
                nc.scalar.activation(et[:], xt[:], AF.Exp,
                                     accum_out=se[:, i:i + 1])
                if USE_MM:
                    rp = ps_r.tile([3, F], f32, tag="rp", name=f"rp_{g}")
                    nc.tensor.matmul(rp[:], w3[:], et[:], start=True, stop=True)
                    m, a = divmod(i, 4)
                    nc.scalar.copy(rstacks[m][32 * a:32 * a + 3, :], rp[:])
                else:
                    dx = dpool.tile([P, F], f32, tag="dx", name=f"dx_{g}")
                    nc.vector.scalar_tensor_tensor(
                        dx[:], et[:], 0.0, xmap[:], AX.add, AX.mult,
                        accum_out=sex[:, i:i + 1])
                    dy = dpool.tile([P, F], f32, tag="dy", name=f"dy_{g}")
                    nc.vector.scalar_tensor_tensor(
                        dy[:], et[:], 0.0, ymap[:], AX.add, AX.mult,
                        accum_out=sey[:, i:i + 1])

                nc.vector.max(tmax[:, 8 * i:8 * i + 8], tt[:])
                nc.vector.max_index(tidx[:, 8 * i:8 * i + 8],
                                    tmax[:, 8 * i:8 * i + 8], tt[:])

            # ---- group combine: S per heatmap -> -ln(S) on all partitions ----
            if USE_MM_COMBINE:
                ps1 = ps_s.tile([1, GRP], f32, tag="ps1", name=f"ps1_{grp}")
                nc.tensor.matmul(ps1[:], ones1[:], se[:], start=True, stop=True)
                lsrow = smpool.tile([1, GRP], f32, tag="lsrow", name=f"lsrow_{grp}")
                nc.scalar.activation(lsrow[:], ps1[:], AF.Ln)
                pbc = ps_b.tile([P, GRP], f32, tag="pbc", name=f"pbc_{grp}")
                nc.tensor.matmul(pbc[:], nones[:], lsrow[:], start=True, stop=True)
                bcnl = smpool.tile([P, GRP], f32, tag="bcnl", name=f"bcnl_{grp}")
                nc.scalar.copy(bcnl[:], pbc[:])
            else:
                sbc = smpool.tile([P, GRP], f32, tag="sbc", name=f"sbc_{grp}")
                nc.gpsimd.partition_all_reduce(sbc[:], se[:], P,
                                               bass_isa.ReduceOp.add)
                bcnl = smpool.tile([P, GRP], f32, tag="bcnl", name=f"bcnl_{grp}")
                nc.scalar.activation(bcnl[:], sbc[:], AF.Ln, scale=1.0)
                nc.scalar.mul(bcnl[:], bcnl[:], -1.0)

            if USE_MM:
                for m in range(2):
                    rdump = dpool.tile([P, F], f32, tag="rdump",
                                       name=f"rdump{m}_{grp}")
                    nc.vector.scalar_tensor_tensor(
                        rdump[:], rstacks[m][:], 0.0, xb[:], AX.add, AX.mult,
                        accum_out=reds[m][:])
                    nc.sync.dma_start(o_red[grp, m], reds[m][:, 0])

            # ---- phase B ----
            for i in range(GRP):
                g = grp * GRP + i
                xt, tt = xts[i], tts[i]
                pt = ppool.tile([P, F], f32, tag="pt", name=f"pt_{g}")
                nc.scalar.activation(pt[:], xt[:], AF.Exp,
                                     bias=bcnl[:, i:i + 1], scale=1.0,
                                     accum_out=sp[:, i:i + 1])
                qt = qpool.tile([P, F], f32, tag="qt", name=f"qt_{g}")
                if USE_STT:
                    nc.vector.scalar_tensor_tensor(
                        qt[:], pt[:], 0.0, tt[:], AX.add, AX.add,
                        accum_out=sq[:, i:i + 1])
                else:
                    nc.vector.tensor_add(qt[:], pt[:], tt[:])
                    nc.vector.tensor_reduce(sq[:, i:i + 1], qt[:],
                                            mybir.AxisListType.X, AX.add)
                lt = lpool.tile([P, F], f32, tag="lt", name=f"lt_{g}")
                nc.scalar.activation(lt[:], qt[:], AF.Ln)
                d1 = dpool.tile([P, F], f32, tag="d1", name=f"d1_{g}")
                if USE_STT:
                    nc.vector.scalar_tensor_tensor(
                        d1[:], qt[:], 0.0, lt[:], AX.add, AX.mult,
                        accum_out=sql[:, i:i + 1])
                    d2 = dpool.tile([P, F], f32, tag="d2", name=f"d2_{g}")
                    nc.vector.scalar_tensor_tensor(
                        d2[:], qt[:], 0.0, qt[:], AX.add, AX.mult,
                        accum_out=sqq[:, i:i + 1])
                else:
                    nc.vector.tensor_mul(d1[:], qt[:], lt[:])
                    nc.vector.tensor_reduce(sql[:, i:i + 1], d1[:],
                                            mybir.AxisListType.X, AX.add)
                    d2 = dpool.tile([P, F], f32, tag="d2", name=f"d2_{g}")
                    nc.vector.tensor_mul(d2[:], qt[:], qt[:])
                    nc.vector.tensor_reduce(sqq[:, i:i + 1], d2[:],
                                            mybir.AxisListType.X, AX.add)

            nc.sync.dma_start(o_se[grp], se[:])
            nc.sync.dma_start(o_sp[grp], sp[:])
            nc.sync.dma_start(o_sq[grp], sq[:])
            nc.sync.dma_start(o_sql[grp], sql[:])
            nc.sync.dma_start(o_sqq[grp], sqq[:])
            nc.sync.dma_start(o_tmax[grp], tmax[:])
            nc.sync.dma_start(o_tidx[grp], tidx[:])
            if not USE_MM:
                nc.sync.dma_start(o_sex[grp], sex[:])
                nc.sync.dma_start(o_sey[grp], sey[:])

    nc.compile()
    return nc


def _get_nc():
    if "nc" not in _CACHE:
        _CACHE["nc"] = _build()
    return _CACHE["nc"]


def _in_maps(inp, tgt):
    consts = _consts()
    maps = []
    for k in range(NCORES):
        xs_k = inp[k * SPC:(k + 1) * SPC].reshape(HM, P, F)
        ts_k = tgt[k * SPC:(k + 1) * SPC].reshape(HM, P, F)
        maps.append({
            "x": np.ascontiguousarray(xs_k),
            "t": np.ascontiguousarray(ts_k),
            **consts,
        })
    return maps


def kernel(input, target):
    from concourse.bass_utils import run_bass_kernel_spmd

    nc = _get_nc()
    inp = np.ascontiguousarray(np.asarray(input, dtype=np.float32))
    tgt = np.ascontiguousarray(np.asarray(target, dtype=np.float32))
    res = run_bass_kernel_spmd(nc, _in_maps(inp, tgt), list(range(NCORES))).results
    return _epilogue(res)


def _epilogue(res):
    # ---------------- host epilogue (float64) ----------------
    pred_x = np.zeros((B, C)); pred_y = np.zeros((B, C))
    true_x = np.zeros((B, C)); true_y = np.zeros((B, C))
    jsd = np.zeros((B, C))

    for k in range(NCORES):
        r = res[k]
        se = r["o_se"].astype(np.float64)          # [NGRP, P, GRP]
        S = se.sum(axis=1).reshape(HM)             # [32]
        if USE_MM:
            red = r["o_red"].astype(np.float64).reshape(NGRP, 2, 4, 32)[..., :3] \
                .reshape(HM, 3)
            sex = red[:, 0]
            sey = red[:, 1] + red[:, 2]
        else:
            sex = r["o_sex"].astype(np.float64).sum(axis=1).reshape(HM)
            sey = r["o_sey"].astype(np.float64).sum(axis=1).reshape(HM)
        sq = r["o_sq"].astype(np.float64).sum(axis=1).reshape(HM)
        sql = r["o_sql"].astype(np.float64).sum(axis=1).reshape(HM)
        sqq = r["o_sqq"].astype(np.float64).sum(axis=1).reshape(HM)
        tmax = r["o_tmax"].reshape(NGRP, P, GRP, 8)[..., 0] \
            .transpose(1, 0, 2).reshape(P, HM)
        tidx = r["o_tidx"].reshape(NGRP, P, GRP, 8)[..., 0] \
            .transpose(1, 0, 2).reshape(P, HM)

        p_star = np.argmax(tmax, axis=0)
        j_star = tidx[p_star, np.arange(HM)].astype(np.int64)
        k_star = 512 * p_star.astype(np.int64) + j_star

        px = (sex / S).reshape(SPC, C)
        py = (sey / S).reshape(SPC, C)
        tx = (((k_star % W).astype(np.float64) + 1.0) - W / 2.0) / W
        ty = (((k_star // W).astype(np.float64) + 1.0) - H / 2.0) / H
        js = ((0.5 * sql - 0.5 * np.log(2.0) * sq - 0.25 * sqq)
              / (H * W)).reshape(SPC, C)

        sl = slice(k * SPC, (k + 1) * SPC)
        pred_x[sl] = px; pred_y[sl] = py
        true_x[sl] = tx.reshape(SPC, C); true_y[sl] = ty.reshape(SPC, C)
        jsd[sl] = js

    ed = np.sqrt((true_x - pred_x) ** 2 + (true_y - pred_y) ** 2)
    s = np.sum(ed + jsd)

    pd = np.sqrt((pred_x[:, 0] - pred_x[:, 1]) ** 2
                 + (pred_y[:, 0] - pred_y[:, 1]) ** 2)
    td = np.sqrt((true_x[:, 0] - true_x[:, 1]) ** 2
                 + (true_y[:, 0] - true_y[:, 1]) ** 2)
    s = s + np.sum(np.abs(pd - td) / td)

    return np.asarray([s / B], dtype=np.float32)
